# revision 5
# baseline (speedup 1.0000x reference)
"""NormLinearAttention Trainium2 kernel (8 NeuronCores, SPMD).

Math (per batch b):
  q = relu(x @ Wq + bq); k = relu(x @ Wk + bk); v = x @ Wv + bv; u = x @ Wu + bu
  kv[h,d,e] = sum_n k[h,n,d] v[h,n,e];  kv = abs_clamp(kv, 0.01, 100)
  a = q @ kv  (per head);  z = LN(a) * ln_w + ln_b;  out = (u * z) @ Wo + bo

Sharding: rows (b, n) flattened to 32768 rows; core c owns rows
[c*4096, (c+1)*4096) — exactly half of batch c//2.  The kv reduction
couples the two halves of each batch: partial kv is AllReduce'd over
core pairs [[0,1],[2,3],[4,5],[6,7]] while the q/u projections run.

On-chip layout: activations are kept feature-major (xT = x transposed
via DMA-transpose on load), so every projection/attention matmul has
its contraction dim on partitions with no on-chip transposes.
q/u are spilled to DRAM scratch between phases to stay inside SBUF.
Compute dtype bf16 (fp32 PSUM accumulation), host pre-casts inputs.
"""

import numpy as np
import ml_dtypes

import concourse.bass as bass
import concourse.mybir as mybir
import concourse.tile as tile
from concourse import bacc
from concourse.bass_utils import run_bass_kernel_spmd

B, N, D, H = 4, 8192, 1024, 16
HD = D // H          # 64
P = 128
DC = D // P          # 8 dim chunks
NCORES = 8
R_FULL = B * N // NCORES   # 4096 rows per core
WIN = 512
EPS = 1e-5
GROUPS = [[0, 1], [2, 3], [4, 5], [6, 7]]

bf16 = mybir.dt.bfloat16
f32 = mybir.dt.float32
AF = mybir.ActivationFunctionType
ALU = mybir.AluOpType
NPBF16 = ml_dtypes.bfloat16


def build(R=R_FULL):
    RT = R // P          # rowtiles
    NW = R // WIN        # windows
    RPW = WIN // P       # rowtiles per window (4)

    nc = bacc.Bacc("TRN2", target_bir_lowering=False, debug=False,
                   enable_asserts=False, num_devices=NCORES)

    x_ext = nc.dram_tensor("x", [R, D], bf16, kind="ExternalInput").ap()
    w_ext = {n: nc.dram_tensor(n, [D, D], bf16, kind="ExternalInput").ap()
             for n in ("wk", "wv", "wq", "wu", "wo")}
    bkb_ext = nc.dram_tensor("bk_b", [P, D], f32, kind="ExternalInput").ap()
    bvb_ext = nc.dram_tensor("bv_b", [P, D], f32, kind="ExternalInput").ap()
    bob_ext = nc.dram_tensor("bo_b", [P, D], f32, kind="ExternalInput").ap()
    bqf_ext = nc.dram_tensor("bq_fm", [P, DC], f32, kind="ExternalInput").ap()
    buf_ext = nc.dram_tensor("bu_fm", [P, DC], f32, kind="ExternalInput").ap()
    lnw_ext = nc.dram_tensor("lnw_row", [1, D], bf16, kind="ExternalInput").ap()
    lnb_ext = nc.dram_tensor("lnb_row", [1, D], bf16, kind="ExternalInput").ap()
    out_ext = nc.dram_tensor("out", [R, D], f32, kind="ExternalOutput").ap()

    with tile.TileContext(nc, num_cores=NCORES) as tc:
        with (
            tc.tile_pool(name="const", bufs=1) as cp,
            tc.tile_pool(name="wpool", bufs=3) as wp,
            tc.tile_pool(name="wps", bufs=6, space="PSUM") as wps,
            tc.tile_pool(name="accps", bufs=1, space="PSUM") as accps,
            tc.tile_pool(name="dram", bufs=1, space="DRAM") as dram,
            tc.tile_pool(name="small", bufs=2) as sp,
        ):
            # ---- constants ----
            ones128 = cp.tile([P, 1], bf16, name="ones128")
            nc.vector.memset(ones128[:], 1.0)
            ones1x = cp.tile([1, WIN], bf16, name="ones1x")
            nc.vector.memset(ones1x[:], 1.0)
            bk_b = cp.tile([P, D], f32, name="bk_b")
            nc.sync.dma_start(bk_b[:], bkb_ext)
            bv_b = cp.tile([P, D], f32, name="bv_b")
            nc.sync.dma_start(bv_b[:], bvb_ext)
            bo_b = cp.tile([P, D], f32, name="bo_b")
            nc.sync.dma_start(bo_b[:], bob_ext)
            bq_fm = cp.tile([P, DC], f32, name="bq_fm")
            nc.sync.dma_start(bq_fm[:], bqf_ext)
            bu_fm = cp.tile([P, DC], f32, name="bu_fm")
            nc.sync.dma_start(bu_fm[:], buf_ext)
            lnw_row = cp.tile([1, D], bf16, name="lnw_row")
            nc.sync.dma_start(lnw_row[:], lnw_ext)
            lnb_row = cp.tile([1, D], bf16, name="lnb_row")
            nc.sync.dma_start(lnb_row[:], lnb_ext)

            # weights, feature-major-ready: w_sb[p, c, n] = W[c*128+p, n]
            w_sb = {}
            for n in ("wk", "wv", "wq", "wu", "wo"):
                t = wp.tile([P, DC, D], bf16, name=f"{n}_sb", tag="W")
                nc.sync.dma_start(t[:], w_ext[n].rearrange("(c p) n -> p c n", p=P))
                w_sb[n] = t

            # kv bounce buffers (pair AllReduce)
            kv_in = dram.tile([P, DC * P], f32, name="kv_in")
            kv_out = dram.tile([P, DC * P], f32, name="kv_out")
            q_dram = dram.tile([P, DC, R], bf16, name="q_dram")
            u_dram = dram.tile([P, DC, R], bf16, name="u_dram")

            kv_blk = sp.tile([P, DC * P], bf16, name="kv_blk", bufs=1)

            with (
                tc.tile_pool(name="xtp", bufs=1) as xtp,
                tc.tile_pool(name="ab", bufs=2) as ab,
            ):
                # ---- load x transposed: xT[p, c, n] = x[n, c*128+p] ----
                xT = xtp.tile([P, DC, R], bf16, name="xT")
                for c in range(DC):
                    nc.sync.dma_start_transpose(xT[:, c, :],
                                                x_ext[:, c * P:(c + 1) * P])

                # ---- phase A: k, v projections + partial kv ----
                kv_ps = accps.tile([P, DC * P], f32, name="kv_ps")
                for rt in range(RT):
                    xTr = xT[:, :, rt * P:(rt + 1) * P]
                    pk0 = wps.tile([P, WIN], f32, name="pk0", tag="work")
                    pk1 = wps.tile([P, WIN], f32, name="pk1", tag="work")
                    pv0 = wps.tile([P, WIN], f32, name="pv0", tag="work")
                    pv1 = wps.tile([P, WIN], f32, name="pv1", tag="work")
                    for c in range(DC):
                        st, sto = c == 0, c == DC - 1
                        lhs = xTr[:, c, :]
                        nc.tensor.matmul(pk0[:], lhs, w_sb["wk"][:, c, 0:WIN],
                                         start=st, stop=sto)
                        nc.tensor.matmul(pk1[:], lhs, w_sb["wk"][:, c, WIN:D],
                                         start=st, stop=sto)
                        nc.tensor.matmul(pv0[:], lhs, w_sb["wv"][:, c, 0:WIN],
                                         start=st, stop=sto)
                        nc.tensor.matmul(pv1[:], lhs, w_sb["wv"][:, c, WIN:D],
                                         start=st, stop=sto)
                    k_bf = ab.tile([P, D], bf16, name="k_bf", tag="kvt", bufs=4)
                    v_bf = ab.tile([P, D], bf16, name="v_bf", tag="kvt", bufs=4)
                    # k = relu(psum + bias) ; v = psum + bias
                    nc.vector.tensor_tensor(pk0[:], pk0[:], bk_b[:, 0:WIN], ALU.add)
                    nc.vector.tensor_tensor(pk1[:], pk1[:], bk_b[:, WIN:D], ALU.add)
                    nc.scalar.activation(k_bf[:, 0:WIN], pk0[:], AF.Relu)
                    nc.scalar.activation(k_bf[:, WIN:D], pk1[:], AF.Relu)
                    nc.vector.tensor_tensor(pv0[:], pv0[:], bv_b[:, 0:WIN], ALU.add)
                    nc.vector.tensor_tensor(pv1[:], pv1[:], bv_b[:, WIN:D], ALU.add)
                    nc.scalar.activation(v_bf[:, 0:WIN], pv0[:], AF.Copy)
                    nc.scalar.activation(v_bf[:, WIN:D], pv1[:], AF.Copy)
                    # partial kv per head-pair: [128,128] block (diag blocks used)
                    for g in range(DC):
                        nc.tensor.matmul(
                            kv_ps[:, g * P:(g + 1) * P],
                            k_bf[:, g * P:(g + 1) * P],
                            v_bf[:, g * P:(g + 1) * P],
                            start=(rt == 0 and g % 4 == 0),
                            stop=(rt == RT - 1 and g % 4 == 3),
                        )

                # kv partial -> DRAM -> pair AllReduce
                kv_sb = sp.tile([P, DC * P], f32, name="kv_sb", bufs=1)
                nc.vector.tensor_copy(kv_sb[:], kv_ps[:])
                nc.sync.dma_start(kv_in[:], kv_sb[:])
                nc.gpsimd.collective_compute(
                    "AllReduce", ALU.add, replica_groups=GROUPS,
                    ins=[kv_in[:]], outs=[kv_out[:]],
                )

                # ---- phase B (overlaps collective): q, u projections ----
                for w in range(NW):
                    xTw = xT[:, :, w * WIN:(w + 1) * WIN]
                    for name, bias, func, dst in (
                        ("wq", bq_fm, AF.Relu, q_dram),
                        ("wu", bu_fm, AF.Identity, u_dram),
                    ):
                        stage = ab.tile([P, DC, WIN], bf16, name=f"{name}_w",
                                        tag="quw", bufs=4)
                        for t in range(DC):
                            ps = wps.tile([P, WIN], f32, name="pqu", tag="work")
                            for c in range(DC):
                                nc.tensor.matmul(
                                    ps[:], w_sb[name][:, c, t * P:(t + 1) * P],
                                    xTw[:, c, :],
                                    start=(c == 0), stop=(c == DC - 1))
                            nc.scalar.activation(stage[:, t, :], ps[:], func,
                                                 bias=bias[:, t:t + 1], scale=1.0)
                        nc.sync.dma_start(dst[:, :, w * WIN:(w + 1) * WIN], stage[:])

            # ---- kv: readback, clamp, build block-diagonal tiles ----
            kv_rb = sp.tile([P, DC * P], f32, name="kv_rb", bufs=1)
            nc.sync.dma_start(kv_rb[:], kv_out[:])
            # clamp to [-100, 100], then |.| >= 0.01 keeping sign
            kv_c1 = sp.tile([P, DC * P], f32, name="kv_c1", bufs=1)
            nc.vector.tensor_scalar(kv_c1[:], kv_rb[:], -100.0, 100.0,
                                    op0=ALU.max, op1=ALU.min)
            kv_sgn = sp.tile([P, DC * P], bf16, name="kv_sgn", bufs=1)
            nc.scalar.activation(kv_sgn[:], kv_c1[:], AF.Sign)
            kv_abs = sp.tile([P, DC * P], f32, name="kv_abs", bufs=1)
            nc.scalar.activation(kv_abs[:], kv_c1[:], AF.Abs)
            nc.vector.tensor_scalar(kv_abs[:], kv_abs[:], 0.01, None, op0=ALU.max)
            kv_cl = sp.tile([P, DC * P], bf16, name="kv_cl", bufs=1)
            nc.vector.tensor_tensor(kv_cl[:], kv_sgn[:], kv_abs[:], ALU.mult)
            # block-diag: kv_blk[0:64, g*128:g*128+64]     = kv(head 2g)
            #             kv_blk[64:128, g*128+64:g*128+128] = kv(head 2g+1)
            nc.vector.memset(kv_blk[:], 0.0)
            for g in range(DC):
                nc.vector.tensor_copy(kv_blk[0:HD, g * P:g * P + HD],
                                      kv_cl[0:HD, g * P:g * P + HD])
                nc.vector.tensor_copy(kv_blk[HD:P, g * P + HD:(g + 1) * P],
                                      kv_cl[HD:P, g * P + HD:(g + 1) * P])

            # ---- phase C: attention, LN, z = u*norm, out = z @ Wo + bo ----
            with tc.tile_pool(name="pc", bufs=2) as pc:
                for w in range(NW):
                    qT_w = pc.tile([P, DC, WIN], bf16, name="qT_w", tag="qtw")
                    nc.sync.dma_start(qT_w[:], q_dram[:, :, w * WIN:(w + 1) * WIN])
                    uT_w = pc.tile([P, DC, WIN], bf16, name="uT_w", tag="utw")
                    nc.sync.dma_start(uT_w[:], u_dram[:, :, w * WIN:(w + 1) * WIN])

                    attn = pc.tile([P, DC, WIN], bf16, name="attn", tag="attn")
                    attn2 = pc.tile([P, DC, WIN], bf16, name="attn2", tag="attn2")
                    for g in range(DC):
                        aps = wps.tile([P, WIN], f32, name="aps", tag="work")
                        nc.tensor.matmul(aps[:], kv_blk[:, g * P:(g + 1) * P],
                                         qT_w[:, g, :], start=True, stop=True)
                        nc.scalar.activation(attn[:, g, :], aps[:], AF.Copy)
                        nc.scalar.activation(attn2[:, g, :], aps[:], AF.Square)

                    # LN stats: per-column sums over all 1024 dims
                    s_ps = wps.tile([1, WIN], f32, name="s_ps", tag="work")
                    q_ps = wps.tile([1, WIN], f32, name="q_ps", tag="work")
                    for g in range(DC):
                        nc.tensor.matmul(s_ps[:], ones128[:], attn[:, g, :],
                                         start=(g == 0), stop=(g == DC - 1))
                        nc.tensor.matmul(q_ps[:], ones128[:], attn2[:, g, :],
                                         start=(g == 0), stop=(g == DC - 1))
                    mean_t = pc.tile([1, WIN], f32, name="mean_t", tag="mean_t")
                    var_t = pc.tile([1, WIN], f32, name="var_t", tag="var_t")
                    nc.vector.tensor_scalar(mean_t[:], s_ps[:], 1.0 / D, None,
                                            op0=ALU.mult)        # mean
                    # var + eps = E[x^2]/1 - mean^2 + eps  (two fused ops)
                    nc.vector.tensor_tensor(var_t[:], mean_t[:], mean_t[:],
                                            ALU.mult)            # mean^2
                    # (q_ps * 1/D - mean^2) + eps:
                    nc.vector.scalar_tensor_tensor(var_t[:], q_ps[:], 1.0 / D,
                                                   var_t[:], ALU.mult,
                                                   ALU.subtract)
                    nc.vector.tensor_scalar(var_t[:], var_t[:], EPS, None,
                                            op0=ALU.add)
                    nc.vector.reciprocal(var_t[:], var_t[:])
                    rstd = pc.tile([1, WIN], bf16, name="rstd", tag="rstd")
                    nc.scalar.activation(rstd[:], var_t[:], AF.Sqrt)
                    shp = pc.tile([1, WIN], bf16, name="shp", tag="shp")
                    # shiftpre = -mean * rstd
                    nc.vector.scalar_tensor_tensor(shp[:], mean_t[:], -1.0,
                                                   rstd[:], ALU.mult, ALU.mult)

                    # z = (attn * (lnw x rstd) + (lnw x shiftpre + lnb x 1)) * u
                    zw = pc.tile([P, DC, WIN], bf16, name="zw", tag="zw")
                    for g in range(DC):
                        sc_ps = wps.tile([P, WIN], f32, name="sc_ps", tag="work")
                        sh_ps = wps.tile([P, WIN], f32, name="sh_ps", tag="work")
                        lw = lnw_row[:, g * P:(g + 1) * P]
                        lb = lnb_row[:, g * P:(g + 1) * P]
                        nc.tensor.matmul(sc_ps[:], lw, rstd[:], start=True, stop=True)
                        nc.tensor.matmul(sh_ps[:], lw, shp[:], start=True, stop=False)
                        nc.tensor.matmul(sh_ps[:], lb, ones1x[:], start=False, stop=True)
                        zt = pc.tile([P, WIN], bf16, name="zt", tag="zt", bufs=3)
                        nc.vector.tensor_tensor(zt[:], attn[:, g, :], sc_ps[:], ALU.mult)
                        nc.vector.tensor_tensor(zt[:], zt[:], sh_ps[:], ALU.add)
                        nc.vector.tensor_tensor(zw[:, g, :], zt[:], uT_w[:, g, :],
                                                ALU.mult)

                    # out = z @ Wo + bo (row-major out, zT chunks stationary)
                    for j in range(RPW):
                        o0 = wps.tile([P, WIN], f32, name="o0", tag="work")
                        o1 = wps.tile([P, WIN], f32, name="o1", tag="work")
                        for c in range(DC):
                            lhs = zw[:, c, j * P:(j + 1) * P]
                            nc.tensor.matmul(o0[:], lhs, w_sb["wo"][:, c, 0:WIN],
                                             start=(c == 0), stop=(c == DC - 1))
                            nc.tensor.matmul(o1[:], lhs, w_sb["wo"][:, c, WIN:D],
                                             start=(c == 0), stop=(c == DC - 1))
                        osb = pc.tile([P, D], f32, name="osb", tag="osb", bufs=3)
                        nc.vector.scalar_tensor_tensor(osb[:, 0:WIN], o0[:], 1.0,
                                                       bo_b[:, 0:WIN],
                                                       ALU.mult, ALU.add)
                        nc.vector.scalar_tensor_tensor(osb[:, WIN:D], o1[:], 1.0,
                                                       bo_b[:, WIN:D],
                                                       ALU.mult, ALU.add)
                        rt = w * RPW + j
                        nc.sync.dma_start(out_ext[rt * P:(rt + 1) * P, :], osb[:])

    nc.compile()
    return nc


def make_in_maps(query, Wq, bq, Wk, bk, Wv, bv, Wu, bu, Wo, bo, ln_w, ln_b,
                 R=R_FULL):
    xs = np.ascontiguousarray(query.reshape(-1, D)).astype(NPBF16)
    common = {
        "wk": np.ascontiguousarray(Wk).astype(NPBF16),
        "wv": np.ascontiguousarray(Wv).astype(NPBF16),
        "wq": np.ascontiguousarray(Wq).astype(NPBF16),
        "wu": np.ascontiguousarray(Wu).astype(NPBF16),
        "wo": np.ascontiguousarray(Wo).astype(NPBF16),
        "bk_b": np.ascontiguousarray(
            np.broadcast_to(bk.astype(np.float32), (P, D))),
        "bv_b": np.ascontiguousarray(
            np.broadcast_to(bv.astype(np.float32), (P, D))),
        "bo_b": np.ascontiguousarray(
            np.broadcast_to(bo.astype(np.float32), (P, D))),
        "bq_fm": np.ascontiguousarray(bq.astype(np.float32).reshape(DC, P).T),
        "bu_fm": np.ascontiguousarray(bu.astype(np.float32).reshape(DC, P).T),
        "lnw_row": np.ascontiguousarray(ln_w.astype(NPBF16).reshape(1, D)),
        "lnb_row": np.ascontiguousarray(ln_b.astype(NPBF16).reshape(1, D)),
    }
    return [dict(common, x=np.ascontiguousarray(xs[c * R:(c + 1) * R]))
            for c in range(NCORES)]


_NC_CACHE = {}


def kernel(query, Wq, bq, Wk, bk, Wv, bv, Wu, bu, Wo, bo, ln_w, ln_b):
    query = np.asarray(query, dtype=np.float32)
    if "nc" not in _NC_CACHE:
        _NC_CACHE["nc"] = build()
    nc = _NC_CACHE["nc"]
    in_maps = make_in_maps(query, np.asarray(Wq), np.asarray(bq),
                           np.asarray(Wk), np.asarray(bk),
                           np.asarray(Wv), np.asarray(bv),
                           np.asarray(Wu), np.asarray(bu),
                           np.asarray(Wo), np.asarray(bo),
                           np.asarray(ln_w), np.asarray(ln_b))
    res = run_bass_kernel_spmd(nc, in_maps, list(range(NCORES)))
    out = np.empty((B * N, D), np.float32)
    for c in range(NCORES):
        out[c * R_FULL:(c + 1) * R_FULL] = res.results[c]["out"]
    return out.reshape(B, N, D)


# revision 32
# speedup vs baseline: 3.4369x; 3.4369x over previous
"""NormLinearAttention Trainium2 kernel (8 NeuronCores, SPMD).

Math (per batch b):
  q = relu(x @ Wq + bq); k = relu(x @ Wk + bk); v = x @ Wv + bv; u = x @ Wu + bu
  kv[h,d,e] = sum_n k[h,n,d] v[h,n,e];  kv = abs_clamp(kv, 0.01, 100)
  a = q @ kv  (per head);  z = LN(a) * ln_w + ln_b;  out = (u * z) @ Wo + bo

Sharding: rows (b, n) flattened to 32768 rows; core c owns rows
[c*4096, (c+1)*4096) — exactly half of batch c//2.  The kv reduction
couples the two halves of each batch: partial kv is AllReduce'd over
core pairs [[0,1],[2,3],[4,5],[6,7]] while the q/u projections run.

On-chip layout: activations are kept feature-major (xT = x transposed
via DMA-transpose on load), so every projection/attention matmul has
its contraction dim on partitions with no on-chip transposes.
q/u are spilled to DRAM scratch between phases to stay inside SBUF.
Compute dtype bf16 (fp32 PSUM accumulation), host pre-casts inputs.
"""

import numpy as np
import ml_dtypes

import concourse.bass as bass
import concourse.mybir as mybir
import concourse.tile as tile
from concourse import bacc
from concourse.bass_utils import run_bass_kernel_spmd

B, N, D, H = 4, 8192, 1024, 16
HD = D // H          # 64
P = 128
DC = D // P          # 8 dim chunks
NCORES = 8
R_FULL = B * N // NCORES   # 4096 rows per core
WIN = 512
EPS = 1e-5
GROUPS = [[0, 1], [2, 3], [4, 5], [6, 7]]
PIPE = 3  # phase C software pipeline depth

bf16 = mybir.dt.bfloat16
f32 = mybir.dt.float32
AF = mybir.ActivationFunctionType
ALU = mybir.AluOpType
NPBF16 = ml_dtypes.bfloat16


def build(R=R_FULL):
    RT = R // P          # rowtiles
    NW = R // WIN        # windows
    RPW = WIN // P       # rowtiles per window (4)

    nc = bacc.Bacc("TRN2", target_bir_lowering=False, debug=False,
                   enable_asserts=False, num_devices=NCORES)

    xt_ext = nc.dram_tensor("xt", [DC, P, R], bf16, kind="ExternalInput").ap()
    w_ext = {n: nc.dram_tensor(n, [D, D], bf16, kind="ExternalInput").ap()
             for n in ("wk", "wv", "wq", "wu", "wo")}
    bkb_ext = nc.dram_tensor("bk_b", [P, D], f32, kind="ExternalInput").ap()
    bvb_ext = nc.dram_tensor("bv_b", [P, D], f32, kind="ExternalInput").ap()
    bob_ext = nc.dram_tensor("bo_b", [P, D], f32, kind="ExternalInput").ap()
    bqf_ext = nc.dram_tensor("bq_fm", [P, DC], f32, kind="ExternalInput").ap()
    buf_ext = nc.dram_tensor("bu_fm", [P, DC], f32, kind="ExternalInput").ap()
    lnw_ext = nc.dram_tensor("lnw_fm", [P, DC], f32, kind="ExternalInput").ap()
    lnb_ext = nc.dram_tensor("lnb_fm", [P, DC], f32, kind="ExternalInput").ap()
    out_ext = nc.dram_tensor("out", [R, D], f32, kind="ExternalOutput").ap()

    with tile.TileContext(nc, num_cores=NCORES) as tc:
        with (
            tc.tile_pool(name="const", bufs=1) as cp,
            tc.tile_pool(name="wop", bufs=1) as wop,
            tc.tile_pool(name="wps", bufs=6, space="PSUM") as wps,
            tc.tile_pool(name="accps", bufs=1, space="PSUM") as accps,
            tc.tile_pool(name="dram", bufs=1, space="DRAM") as dram,
            tc.tile_pool(name="small", bufs=2) as sp,
        ):
            # ---- constants ----
            ones128 = cp.tile([P, 1], bf16, name="ones128")
            nc.vector.memset(ones128[:], 1.0)

            # kv bounce buffers (pair AllReduce)
            kv_in = dram.tile([P, DC * P], f32, name="kv_in")
            kv_out = dram.tile([P, DC * P], f32, name="kv_out")
            q_dram = dram.tile([P, DC, R], bf16, name="q_dram")
            u_dram = dram.tile([P, DC, R], bf16, name="u_dram")

            kv_blk = sp.tile([P, DC * P], bf16, name="kv_blk", bufs=1)

            with (
                tc.tile_pool(name="xtp", bufs=1) as xtp,
                tc.tile_pool(name="ab", bufs=2) as ab,
                tc.tile_pool(name="wpool", bufs=3) as wp,
            ):
                # weights first (phase A's first matmuls need wk/wv plus only
                # their own xT chunk), then xT chunks in consumption order
                w_sb = {}
                for n in ("wk", "wv"):
                    t = wp.tile([P, DC, D], bf16, name=f"{n}_sb", tag="W")
                    nc.sync.dma_start(t[:],
                                      w_ext[n].rearrange("(c p) n -> p c n", p=P))
                    w_sb[n] = t

                # ---- xT chunks (host provides x pre-transposed) ----
                xT = [xtp.tile([P, R], bf16, name=f"xT{c}", tag=f"xT{c}")
                      for c in range(DC)]
                for c in range(DC):
                    nc.sync.dma_start(xT[c][:], xt_ext[c])

                # biases and LN params (first needed ~10us in, after xT0)
                bk_b = cp.tile([P, D], f32, name="bk_b")
                nc.sync.dma_start(bk_b[:], bkb_ext)
                bv_b = cp.tile([P, D], f32, name="bv_b")
                nc.sync.dma_start(bv_b[:], bvb_ext)
                bo_b = cp.tile([P, D], f32, name="bo_b")
                nc.sync.dma_start(bo_b[:], bob_ext)
                bq_fm = cp.tile([P, DC], f32, name="bq_fm")
                nc.sync.dma_start(bq_fm[:], bqf_ext)
                bu_fm = cp.tile([P, DC], f32, name="bu_fm")
                nc.sync.dma_start(bu_fm[:], buf_ext)
                lnw_fm = cp.tile([P, DC], f32, name="lnw_fm")
                nc.sync.dma_start(lnw_fm[:], lnw_ext)
                lnb_fm = cp.tile([P, DC], f32, name="lnb_fm")
                nc.sync.dma_start(lnb_fm[:], lnb_ext)

                for n in ("wq", "wu"):
                    t = wp.tile([P, DC, D], bf16, name=f"{n}_sb", tag="W")
                    nc.sync.dma_start(t[:],
                                      w_ext[n].rearrange("(c p) n -> p c n", p=P))
                    w_sb[n] = t
                t = wop.tile([P, DC, D], bf16, name="wo_sb")
                nc.sync.dma_start(t[:],
                                  w_ext["wo"].rearrange("(c p) n -> p c n", p=P))
                w_sb["wo"] = t

                # ---- phase A: k, v projections + partial kv ----
                kv_ps = accps.tile([P, DC * P], f32, name="kv_ps")
                for rt in range(RT):
                    pk0 = wps.tile([P, WIN], f32, name="pk0", tag="work")
                    pk1 = wps.tile([P, WIN], f32, name="pk1", tag="work")
                    pv0 = wps.tile([P, WIN], f32, name="pv0", tag="work")
                    pv1 = wps.tile([P, WIN], f32, name="pv1", tag="work")
                    for c in range(DC):
                        st, sto = c == 0, c == DC - 1
                        lhs = xT[c][:, rt * P:(rt + 1) * P]
                        nc.tensor.matmul(pk0[:], lhs, w_sb["wk"][:, c, 0:WIN],
                                         start=st, stop=sto)
                        nc.tensor.matmul(pk1[:], lhs, w_sb["wk"][:, c, WIN:D],
                                         start=st, stop=sto)
                        nc.tensor.matmul(pv0[:], lhs, w_sb["wv"][:, c, 0:WIN],
                                         start=st, stop=sto)
                        nc.tensor.matmul(pv1[:], lhs, w_sb["wv"][:, c, WIN:D],
                                         start=st, stop=sto)
                    k_bf = ab.tile([P, D], bf16, name="k_bf", tag="kvt", bufs=4)
                    v_bf = ab.tile([P, D], bf16, name="v_bf", tag="kvt", bufs=4)
                    # k = relu(psum + bias) ; v = psum + bias
                    nc.vector.tensor_tensor(pk0[:], pk0[:], bk_b[:, 0:WIN], ALU.add)
                    nc.vector.tensor_tensor(pk1[:], pk1[:], bk_b[:, WIN:D], ALU.add)
                    nc.scalar.activation(k_bf[:, 0:WIN], pk0[:], AF.Relu)
                    nc.scalar.activation(k_bf[:, WIN:D], pk1[:], AF.Relu)
                    nc.vector.tensor_tensor(pv0[:], pv0[:], bv_b[:, 0:WIN], ALU.add)
                    nc.vector.tensor_tensor(pv1[:], pv1[:], bv_b[:, WIN:D], ALU.add)
                    nc.scalar.activation(v_bf[:, 0:WIN], pv0[:], AF.Copy)
                    nc.scalar.activation(v_bf[:, WIN:D], pv1[:], AF.Copy)
                    # partial kv per head-pair: [128,128] block (diag blocks used)
                    for g in range(DC):
                        nc.tensor.matmul(
                            kv_ps[:, g * P:(g + 1) * P],
                            k_bf[:, g * P:(g + 1) * P],
                            v_bf[:, g * P:(g + 1) * P],
                            start=(rt == 0 and g % 4 == 0),
                            stop=(rt == RT - 1 and g % 4 == 3),
                        )

                # kv partial -> DRAM -> pair AllReduce
                kv_sb = sp.tile([P, DC * P], f32, name="kv_sb", bufs=1)
                nc.vector.tensor_copy(kv_sb[:], kv_ps[:])
                nc.sync.dma_start(kv_in[:], kv_sb[:])
                nc.gpsimd.collective_compute(
                    "AllReduce", ALU.add, replica_groups=GROUPS,
                    ins=[kv_in[:]], outs=[kv_out[:]],
                )

                # ---- phase B (overlaps collective): q, u projections ----
                for w in range(NW):
                    for name, bias, func, dst in (
                        ("wq", bq_fm, AF.Relu, q_dram),
                        ("wu", bu_fm, AF.Identity, u_dram),
                    ):
                        stage = ab.tile([P, DC, WIN], bf16, name=f"{name}_w",
                                        tag="quw", bufs=3)
                        for t in range(DC):
                            ps = wps.tile([P, WIN], f32, name="pqu", tag="work")
                            for c in range(DC):
                                nc.tensor.matmul(
                                    ps[:], w_sb[name][:, c, t * P:(t + 1) * P],
                                    xT[c][:, w * WIN:(w + 1) * WIN],
                                    start=(c == 0), stop=(c == DC - 1))
                            nc.scalar.activation(stage[:, t, :], ps[:], func,
                                                 bias=bias[:, t:t + 1], scale=1.0)
                        nc.sync.dma_start(dst[:, :, w * WIN:(w + 1) * WIN], stage[:])

            # ---- kv: readback, clamp, build block-diagonal tiles ----
            kv_rb = sp.tile([P, DC * P], f32, name="kv_rb", bufs=1)
            nc.sync.dma_start(kv_rb[:], kv_out[:])
            # clamp to [-100, 100], then |.| >= 0.01 keeping sign (in-place)
            nc.vector.tensor_scalar(kv_rb[:], kv_rb[:], -100.0, 100.0,
                                    op0=ALU.max, op1=ALU.min)
            kv_sgn = sp.tile([P, DC * P], bf16, name="kv_sgn", bufs=1)
            nc.scalar.activation(kv_sgn[:], kv_rb[:], AF.Sign)
            nc.scalar.activation(kv_rb[:], kv_rb[:], AF.Abs)
            nc.vector.tensor_scalar(kv_rb[:], kv_rb[:], 0.01, None, op0=ALU.max)
            kv_cl = sp.tile([P, DC * P], bf16, name="kv_cl", bufs=1)
            nc.vector.tensor_tensor(kv_cl[:], kv_sgn[:], kv_rb[:], ALU.mult)
            # block-diag: kv_blk[0:64, g*128:g*128+64]     = kv(head 2g)
            #             kv_blk[64:128, g*128+64:g*128+128] = kv(head 2g+1)
            nc.vector.memset(kv_blk[:], 0.0)
            for g in range(DC):
                nc.vector.tensor_copy(kv_blk[0:HD, g * P:g * P + HD],
                                      kv_cl[0:HD, g * P:g * P + HD])
                nc.vector.tensor_copy(kv_blk[HD:P, g * P + HD:(g + 1) * P],
                                      kv_cl[HD:P, g * P + HD:(g + 1) * P])

            # ---- phase C: attention, LN, z = u*norm, out = z @ Wo + bo ----
            # Software-pipelined depth 2: out2 matmuls for window w-2 are
            # issued after attn/stats matmuls of window w, so the PE queue
            # never head-of-line blocks on window w's LN chain (DVE/ACT/Pool).
            with tc.tile_pool(name="pc", bufs=2) as pc:
                zw_tiles = {}
                for w in range(NW + PIPE):
                  if w < NW:
                    qT_w = pc.tile([P, DC, WIN], bf16, name="qT_w", tag="qtw")
                    nc.sync.dma_start(qT_w[:], q_dram[:, :, w * WIN:(w + 1) * WIN])
                    uT_w = pc.tile([P, DC, WIN], bf16, name="uT_w", tag="utw")
                    nc.sync.dma_start(uT_w[:], u_dram[:, :, w * WIN:(w + 1) * WIN])

                    attn = pc.tile([P, DC, WIN], bf16, name="attn", tag="attn")
                    attn2 = pc.tile([P, DC, WIN], bf16, name="attn2", tag="attn2")
                    for g in range(DC):
                        aps = wps.tile([P, WIN], f32, name="aps", tag="work")
                        nc.tensor.matmul(aps[:], kv_blk[:, g * P:(g + 1) * P],
                                         qT_w[:, g, :], start=True, stop=True)
                        nc.scalar.activation(attn[:, g, :], aps[:], AF.Copy)
                        nc.scalar.activation(attn2[:, g, :], aps[:], AF.Square)

                    # LN stats: per-column sums over all 1024 dims
                    s_ps = wps.tile([1, WIN], f32, name="s_ps", tag="work")
                    q_ps = wps.tile([1, WIN], f32, name="q_ps", tag="work")
                    for g in range(DC):
                        nc.tensor.matmul(s_ps[:], ones128[:], attn[:, g, :],
                                         start=(g == 0), stop=(g == DC - 1))
                        nc.tensor.matmul(q_ps[:], ones128[:], attn2[:, g, :],
                                         start=(g == 0), stop=(g == DC - 1))
                    mean_t = pc.tile([1, WIN], f32, name="mean_t", tag="mean_t")
                    var_t = pc.tile([1, WIN], f32, name="var_t", tag="var_t")
                    nc.vector.tensor_scalar(mean_t[:], s_ps[:], 1.0 / D, None,
                                            op0=ALU.mult)        # mean
                    # var + eps = E[x^2]/1 - mean^2 + eps  (two fused ops)
                    nc.vector.tensor_tensor(var_t[:], mean_t[:], mean_t[:],
                                            ALU.mult)            # mean^2
                    # (q_ps * 1/D - mean^2) + eps:
                    nc.vector.scalar_tensor_tensor(var_t[:], q_ps[:], 1.0 / D,
                                                   var_t[:], ALU.mult,
                                                   ALU.subtract)
                    nc.vector.tensor_scalar(var_t[:], var_t[:], EPS, None,
                                            op0=ALU.add)
                    nc.vector.reciprocal(var_t[:], var_t[:])
                    rstd = pc.tile([1, WIN], bf16, name="rstd", tag="rstd")
                    nc.scalar.activation(rstd[:], var_t[:], AF.Sqrt)
                    shp = pc.tile([1, WIN], bf16, name="shp", tag="shp")
                    # shiftpre = -mean * rstd
                    nc.vector.scalar_tensor_tensor(shp[:], mean_t[:], -1.0,
                                                   rstd[:], ALU.mult, ALU.mult)
                    # broadcast per-column stats to all partitions (GPSIMD)
                    rstd_b = pc.tile([P, WIN], bf16, name="rstd_b", tag="rstd_b")
                    nc.gpsimd.partition_broadcast(rstd_b[:], rstd[:])
                    shp_b = pc.tile([P, WIN], bf16, name="shp_b", tag="shp_b")
                    nc.gpsimd.partition_broadcast(shp_b[:], shp[:])

                    # z = ((attn * rstd + shiftpre) * lnw + lnb) * u
                    zw = pc.tile([P, DC, WIN], bf16, name="zw", tag="zw", bufs=PIPE + 1)
                    for g in range(DC):
                        zt = pc.tile([P, WIN], bf16, name="zt", tag="zt", bufs=3)
                        nc.vector.tensor_tensor(zt[:], attn[:, g, :], rstd_b[:],
                                                ALU.mult)
                        nc.vector.tensor_tensor(zt[:], zt[:], shp_b[:], ALU.add)
                        nc.vector.tensor_scalar(zt[:], zt[:], lnw_fm[:, g:g + 1],
                                                lnb_fm[:, g:g + 1],
                                                op0=ALU.mult, op1=ALU.add)
                        nc.vector.tensor_tensor(zw[:, g, :], zt[:], uT_w[:, g, :],
                                                ALU.mult)
                    zw_tiles[w] = zw

                  if w >= PIPE:
                    # out = z @ Wo + bo for window w-PIPE (row-major out,
                    # zT chunks stationary)
                    wc = w - PIPE
                    zw = zw_tiles.pop(wc)
                    for j in range(RPW):
                        o0 = wps.tile([P, WIN], f32, name="o0", tag="work")
                        o1 = wps.tile([P, WIN], f32, name="o1", tag="work")
                        for c in range(DC):
                            lhs = zw[:, c, j * P:(j + 1) * P]
                            nc.tensor.matmul(o0[:], lhs, w_sb["wo"][:, c, 0:WIN],
                                             start=(c == 0), stop=(c == DC - 1))
                            nc.tensor.matmul(o1[:], lhs, w_sb["wo"][:, c, WIN:D],
                                             start=(c == 0), stop=(c == DC - 1))
                        osb = pc.tile([P, D], f32, name="osb", tag="osb", bufs=3)
                        nc.vector.scalar_tensor_tensor(osb[:, 0:WIN], o0[:], 1.0,
                                                       bo_b[:, 0:WIN],
                                                       ALU.mult, ALU.add)
                        nc.vector.scalar_tensor_tensor(osb[:, WIN:D], o1[:], 1.0,
                                                       bo_b[:, WIN:D],
                                                       ALU.mult, ALU.add)
                        rt = wc * RPW + j
                        nc.sync.dma_start(out_ext[rt * P:(rt + 1) * P, :], osb[:])

    nc.compile()
    return nc


def make_in_maps(query, Wq, bq, Wk, bk, Wv, bv, Wu, bu, Wo, bo, ln_w, ln_b,
                 R=R_FULL):
    xs = query.reshape(-1, D).astype(NPBF16)
    common = {
        "wk": np.ascontiguousarray(Wk).astype(NPBF16),
        "wv": np.ascontiguousarray(Wv).astype(NPBF16),
        "wq": np.ascontiguousarray(Wq).astype(NPBF16),
        "wu": np.ascontiguousarray(Wu).astype(NPBF16),
        "wo": np.ascontiguousarray(Wo).astype(NPBF16),
        "bk_b": np.ascontiguousarray(
            np.broadcast_to(bk.astype(np.float32), (P, D))),
        "bv_b": np.ascontiguousarray(
            np.broadcast_to(bv.astype(np.float32), (P, D))),
        "bo_b": np.ascontiguousarray(
            np.broadcast_to(bo.astype(np.float32), (P, D))),
        "bq_fm": np.ascontiguousarray(bq.astype(np.float32).reshape(DC, P).T),
        "bu_fm": np.ascontiguousarray(bu.astype(np.float32).reshape(DC, P).T),
        "lnw_fm": np.ascontiguousarray(ln_w.astype(np.float32).reshape(DC, P).T),
        "lnb_fm": np.ascontiguousarray(ln_b.astype(np.float32).reshape(DC, P).T),
    }
    return [dict(common, xt=np.ascontiguousarray(
                xs[c * R:(c + 1) * R].T.reshape(DC, P, R)))
            for c in range(NCORES)]


_NC_CACHE = {}


def kernel(query, Wq, bq, Wk, bk, Wv, bv, Wu, bu, Wo, bo, ln_w, ln_b):
    query = np.asarray(query, dtype=np.float32)
    if "nc" not in _NC_CACHE:
        _NC_CACHE["nc"] = build()
    nc = _NC_CACHE["nc"]
    in_maps = make_in_maps(query, np.asarray(Wq), np.asarray(bq),
                           np.asarray(Wk), np.asarray(bk),
                           np.asarray(Wv), np.asarray(bv),
                           np.asarray(Wu), np.asarray(bu),
                           np.asarray(Wo), np.asarray(bo),
                           np.asarray(ln_w), np.asarray(ln_b))
    res = run_bass_kernel_spmd(nc, in_maps, list(range(NCORES)))
    out = np.empty((B * N, D), np.float32)
    for c in range(NCORES):
        out[c * R_FULL:(c + 1) * R_FULL] = res.results[c]["out"]
    return out.reshape(B, N, D)
